# revision 11
# baseline (speedup 1.0000x reference)
"""Trainium2 8-core kernel for nn_Attn_user_47863115547245.

reference:
    proj     = id_emb @ attn_W.T + attn_b                  # [seq, hid]
    energies = w1*(user @ proj.T) + w2*(socail @ proj.T)   # [state, seq]
    out      = softmax(energies, axis=-1)

Algebraic restructuring (exact up to float rounding):
  * linearity: energies = (w1*user + w2*socail) @ proj.T
  * reassociation: combined @ (W @ id.T) == (combined @ W) @ id.T, and
    state(2048) < seq(4096) makes (combined @ W) first strictly cheaper.
  * the bias term contributes combined_i . b, constant along the softmax
    axis -> cancels exactly in softmax; dropped.
  * w_big = the larger of |w1|,|w2| is folded into W on the host;
    the ratio w_small/w_big is applied on-device in the combine step.

Sharding: data-parallel over state rows, 256 rows/core x 8 cores.
id_emb (fp16, pre-transposed, s-block-packed) and W (fp16, pre-scaled)
replicated. Softmax is row-local -> zero collectives.

v2 schedule (from v1 NTFF profile analysis):
  * PE effective clock here is ~2.0 GHz sustained (P0), so mm1+mm2
    streaming (81920 cols) is ~41us of PE busy -- the kernel is
    PE-bound. Everything else must hide behind it.
  * v1 lost ~7us to a 4.4us PE idle gap after warmup (HAM re-throttled
    to 1.2GHz, mm1 restarted cold) and ~4us starting mm1 only after
    3MB of W/u/s landed. v2 streams W+us in 8 h-pair bundles raced
    with mm1's h-outer loop, and emits enough warmup matmuls to keep
    the PE busy until the first bundle lands.
  * v1 spent 10.9us in a post-matmul tail (all 16 normalize+out-DMA
    chunks after the last matmul). v2 finishes m0's softmax chain
    under m1's mm2 and splits m1's out-DMA across two rings, cutting
    the tail to ~4-5us.
  * DMA rings: sync carries W + even id chunks, scalar carries u/s +
    odd id chunks, gpsimd carries m0's output, sync/scalar share m1's.
"""

import numpy as np

STATE, SEQ, HID = 2048, 4096, 1024
NCORES = 8
ROWS = STATE // NCORES        # 256 state rows per core
P = 128                       # partitions
KT = HID // P                 # 8 contraction tiles
MT = ROWS // P                # 2 output row tiles per core
SB = 512                      # seq block (one fp32 PSUM bank)
ST = SEQ // SB                # 8 seq blocks
WARM512 = 8                   # PE warmup matmuls, 512-col (HAM fuel)
WARM256 = 6                   # fine-grained warmup tail

_graph_cache: dict = {}


def _build(ratio: float, swap: bool):
    """Build the per-core Bass graph.

    cT[k]     = us[k,0]*ratio + us[k,1]                 (fp16, DVE)
    tmpT[k,m] = sum_h W'[h,k] * cT[h,m]                 (mm1, h-outer)
    E[m,s]    = sum_k tmpT[k,m] * idT[k,s]              (mm2, s-outer)
    out[m,s]  = softmax_s(E)                            (per-chunk online)
    """
    import concourse.bacc as bacc
    import concourse.mybir as mybir
    import concourse.bass as bass
    from concourse import tile

    f32, f16 = mybir.dt.float32, mybir.dt.float16
    AX = mybir.AxisListType.X
    ALU = mybir.AluOpType
    ACTF = mybir.ActivationFunctionType

    nc = bacc.Bacc()

    usT = nc.declare_dram_parameter("usT", [P, KT, 2, ROWS], f16, isOutput=False)
    Wp = nc.declare_dram_parameter("Wp", [P, KT, HID], f16, isOutput=False)
    idT = nc.declare_dram_parameter("idT", [ST, P, KT, SB], f16, isOutput=False)
    out = nc.declare_dram_parameter("out", [ROWS, SEQ], f16, isOutput=True)

    with tile.TileContext(nc) as tc:
        with (
            tc.tile_pool(name="sb", bufs=1) as work,
            tc.tile_pool(name="psum", bufs=1,
                         space=bass.MemorySpace.PSUM) as psp,
        ):
            inp = work
            # ---- warmup: garbage matmuls keep the PE busy (HAM at 8/8)
            # until mm1's first DMA bundle lands. wgarb memset is the
            # only cross-engine dep; gpsimd is free this early.
            wgarb = work.tile([P, SB], f16, tag="warmgarb")
            nc.gpsimd.memset(wgarb[:], 0.0)
            # mm1 accumulator: 4 banks, kb pairs packed side by side.
            psA = psp.tile([P, KT // 2, SB], f32, tag="mm1acc")
            for _ in range(WARM512):
                nc.tensor.matmul(
                    psA[:, 0, :], wgarb[:, :P], wgarb[:],
                    start=True, stop=True)
            for _ in range(WARM256):
                nc.tensor.matmul(
                    psA[:, 0, :ROWS], wgarb[:, :P], wgarb[:, :ROWS],
                    start=True, stop=True)

            # ---- input DMAs ----
            # The DMA engines are descriptor-rate bound (~270ns per 4KB
            # per-partition run per engine), so fewer/bigger contiguous
            # runs win: W ships as two 1MB halves (8KB runs), one per
            # ring. us k-quarters (2KB runs) lead on sync so the first
            # combines are ready before W lands. The sync ring's data
            # flow starts ~2us before the scalar ring's (measured).
            W_sb = inp.tile([P, KT, HID], f16)
            us_sb = inp.tile([P, KT, 2, ROWS], f16)
            id_sb = inp.tile([P, ST, KT, SB], f16)

            H2 = KT // 2
            nc.sync.dma_start(us_sb[:, 0:2, :, :], usT[:, 0:2, :, :])
            nc.scalar.dma_start(us_sb[:, 2:H2, :, :], usT[:, 2:H2, :, :])
            nc.sync.dma_start(W_sb[:, :H2, :], Wp[:, :H2, :])
            nc.scalar.dma_start(W_sb[:, H2:, :], Wp[:, H2:, :])
            nc.sync.dma_start(us_sb[:, H2:, :, :], usT[:, H2:, :, :])
            for s in range(ST):
                eng = nc.sync if s % 2 == 0 else nc.scalar
                eng.dma_start(id_sb[:, s, :, :], idT[s])

            # combine: cT[k] = us[k,0] * ratio + us[k,1], fp16
            cT_sb = work.tile([P, KT, ROWS], f16)
            for k in range(KT):
                nc.vector.scalar_tensor_tensor(
                    cT_sb[:, k, :], us_sb[:, k, 0, :], float(ratio),
                    us_sb[:, k, 1, :], op0=ALU.mult, op1=ALU.add,
                )

            # ---- mm1: two half-passes (4 kb accumulation groups per
            # pass, one full PSUM bank each -- the sim forbids two
            # pending groups in one bank). Each pass is h-outer, so
            # pass 0 races the W/us DMA bundles; pass 1 replays h from
            # SBUF at full PE speed.
            tmpT_sb = work.tile([P, KT, ROWS], f16)
            for half in range(2):
                kb0 = half * (KT // 2)
                for h in range(KT):
                    for kb in range(KT // 2):
                        nc.tensor.matmul(
                            psA[:, kb, :ROWS],
                            W_sb[:, h, P * (kb0 + kb):P * (kb0 + kb + 1)],
                            cT_sb[:, h, :],
                            start=(h == 0), stop=(h == KT - 1),
                        )
                nc.vector.tensor_copy(
                    tmpT_sb[:, kb0:kb0 + KT // 2, :], psA[:, :, :ROWS])

            # ---- mm2 (s-chunk outer, m inner) + per-chunk online softmax
            # negMh holds the NEGATED per-chunk maxes (what reduce_max
            # negate=True emits and what the exp bias wants); m1 gets an
            # extra slot because its final bank is split 2x256 to
            # shorten the post-last-matmul max+exp chain.
            NCH = [ST, ST + 1]
            pun_sb = work.tile([P, MT, SEQ], f16)
            negMh = [work.tile([P, NCH[m]], f32, tag=f"Mh{m}", name=f"Mh{m}")
                     for m in range(MT)]
            Sh = [work.tile([P, NCH[m]], f32, tag=f"Sh{m}", name=f"Sh{m}")
                  for m in range(MT)]
            ehrs = {}

            def rescale(m):
                """ehr[slot] = exp(Mh[slot]-Mtot)/stot (small ops)."""
                n = NCH[m]
                negmtot = work.tile([P, 1], f32, tag=f"negmtot{m}",
                                    name=f"negmtot{m}")
                nc.vector.tensor_reduce(
                    negmtot[:], negMh[m][:], axis=AX, op=ALU.min)
                eh = work.tile([P, n], f32, tag=f"eh{m}", name=f"eh{m}")
                nc.scalar.activation(
                    eh[:], negMh[m][:], ACTF.Exp, bias=negmtot[:], scale=-1.0)
                sehs = work.tile([P, n], f32, tag=f"sehs{m}", name=f"sehs{m}")
                nc.vector.tensor_mul(sehs[:], Sh[m][:], eh[:])
                stot = work.tile([P, 1], f32, tag=f"stot{m}", name=f"stot{m}")
                nc.vector.reduce_sum(stot[:], sehs[:], axis=AX)
                rinv = work.tile([P, 1], f32, tag=f"rinv{m}", name=f"rinv{m}")
                nc.vector.reciprocal(rinv[:], stot[:])
                ehr = work.tile([P, n], f32, tag=f"ehr{m}", name=f"ehr{m}")
                nc.vector.tensor_scalar_mul(ehr[:], eh[:], rinv[:])
                ehrs[m] = ehr

            def norm_chunk(m, s, cols, slot, eng):
                lo = SB * s
                if eng is nc.scalar:
                    nc.scalar.activation(
                        pun_sb[:, m, lo:lo + cols], pun_sb[:, m, lo:lo + cols],
                        ACTF.Copy, scale=ehrs[m][:, slot:slot + 1])
                else:
                    eng.tensor_scalar_mul(
                        pun_sb[:, m, lo:lo + cols], pun_sb[:, m, lo:lo + cols],
                        ehrs[m][:, slot:slot + 1])

            def out_pair(m, s0, eng):
                # one dma_start per 2 chunks: [128, 1024] = 2KB/row runs
                eng.dma_start(
                    out[P * m:P * (m + 1), SB * s0:SB * (s0 + 2)],
                    pun_sb[:, m, SB * s0:SB * (s0 + 2)])

            for s in range(ST):
                for m in range(MT):
                    last = (s == ST - 1 and m == 1)
                    nsub = 2 if last else 1      # split the final bank 2x256
                    for sub in range(nsub):
                        cols = SB // nsub
                        ps2 = psp.tile([P, SB], f32, tag="ps", bufs=4)
                        for k in range(KT):
                            nc.tensor.matmul(
                                ps2[:, :cols],
                                tmpT_sb[:, k, P * m:P * (m + 1)],
                                id_sb[:, s, k, sub * cols:sub * cols + cols],
                                start=(k == 0), stop=(k == KT - 1),
                            )
                        # pun columns start at SB*s + sub*cols
                        nc_slot = s + sub
                        nc.vector.reduce_max(
                            negMh[m][:, nc_slot:nc_slot + 1], ps2[:, :cols],
                            axis=AX, negate=True)
                        nc.scalar.activation(
                            pun_sb[:, m, SB * s + sub * cols:
                                   SB * s + (sub + 1) * cols],
                            ps2[:, :cols], ACTF.Exp,
                            bias=negMh[m][:, nc_slot:nc_slot + 1], scale=1.0,
                            accum_out=Sh[m][:, nc_slot:nc_slot + 1],
                        )
                    if s == ST - 1 and m == 0:
                        # m0 complete: compute its rescale factors now so
                        # the norms (emitted below, after rescale(1) so
                        # eh1 isn't stuck behind COPY norms in the ACT
                        # queue) can start under m1's final chunk.
                        rescale(0)

            rescale(1)
            # normalize: DVE single-op multiplies are ~3x faster than ACT
            # COPY, so DVE takes 6 chunks per m, ACT 2.
            for ss in range(6):
                norm_chunk(0, ss, SB, ss, nc.vector)
            norm_chunk(0, 6, SB, 6, nc.scalar)
            norm_chunk(0, 7, SB, 7, nc.scalar)
            out_pair(0, 0, nc.gpsimd)
            out_pair(0, 2, nc.sync)
            out_pair(0, 4, nc.scalar)
            out_pair(0, 6, nc.gpsimd)
            for ss in range(6):
                norm_chunk(1, ss, SB, ss, nc.vector)
            norm_chunk(1, 7, SB // 2, 7, nc.vector)      # split bank halves
            nc.vector.tensor_scalar_mul(
                pun_sb[:, 1, SB * 7 + SB // 2:SB * 8],
                pun_sb[:, 1, SB * 7 + SB // 2:SB * 8],
                ehrs[1][:, 8:9])
            norm_chunk(1, 6, SB, 6, nc.scalar)
            out_pair(1, 0, nc.sync)
            out_pair(1, 2, nc.scalar)
            out_pair(1, 4, nc.gpsimd)
            out_pair(1, 6, nc.sync)

    nc.compile()
    return nc


def _prepare(user_emb, id_emb, socail_uid_emb, attn_W, w1, w2):
    """Host-side sharding + packing. Returns (ratio, swap, in_maps).

    Packed layouts (per-partition contiguous runs -> few big DMA
    descriptors):
      usT: [128, KT, 2, ROWS]  [p,k,0,m] = in0[rows0+m, k*128+p]
                               [p,k,1,m] = in1[rows0+m, k*128+p]  (fp16)
      Wp:  [128, KT, HID]      [p,h,c] = wbig*W[h*128+p, c]       (fp16)
      idT: [ST, 128, KT, SB]   [s,p,k,c] = id[s*512+c, k*128+p]   (fp16)
    where in0 is the smaller-|w| side (scaled by ratio on device) and
    in1 the larger side.
    """
    w1 = float(np.asarray(w1))
    w2 = float(np.asarray(w2))
    swap = abs(w2) > abs(w1)
    wbig = w2 if swap else w1
    wsmall = w1 if swap else w2
    ratio = (wsmall / wbig) if wbig != 0.0 else 0.0

    Wp = (np.float32(wbig) * np.asarray(attn_W, np.float32)).astype(np.float16)
    Wp_pack = np.ascontiguousarray(Wp.reshape(KT, P, HID).transpose(1, 0, 2))

    idh = np.asarray(id_emb, np.float32).astype(np.float16)      # [SEQ, HID]
    idT_pack = np.ascontiguousarray(
        idh.reshape(ST, SB, KT, P).transpose(0, 3, 2, 1)         # [s,p,k,c]
    )

    u = np.asarray(user_emb, np.float32).astype(np.float16)
    s_ = np.asarray(socail_uid_emb, np.float32).astype(np.float16)
    in0_full = s_ if not swap else u       # scaled by ratio on device
    in1_full = u if not swap else s_

    in_maps = []
    for i in range(NCORES):
        rows = slice(ROWS * i, ROWS * (i + 1))
        us = np.empty((P, KT, 2, ROWS), np.float16)
        us[:, :, 0, :] = in0_full[rows].reshape(ROWS, KT, P).transpose(2, 1, 0)
        us[:, :, 1, :] = in1_full[rows].reshape(ROWS, KT, P).transpose(2, 1, 0)
        in_maps.append({
            "usT": np.ascontiguousarray(us),
            "Wp": Wp_pack,
            "idT": idT_pack,
        })
    return ratio, swap, in_maps


def kernel(user_emb, id_emb, socail_uid_emb, attn_W, attn_b, w1, w2):
    from concourse.bass_utils import run_bass_kernel_spmd

    ratio, swap, in_maps = _prepare(user_emb, id_emb, socail_uid_emb, attn_W, w1, w2)

    key = (round(ratio, 9), swap)
    nc = _graph_cache.get(key)
    if nc is None:
        nc = _build(ratio, swap)
        _graph_cache[key] = nc

    res = run_bass_kernel_spmd(nc, in_maps, core_ids=list(range(NCORES)))
    return np.concatenate(
        [res.results[i]["out"].astype(np.float32) for i in range(NCORES)], axis=0)
